# revision 1
# baseline (speedup 1.0000x reference)
"""CatAttention forward for Trainium2, data-parallel over batch on 8 NeuronCores.

Reference math (B=64, S=2048, D=128, DV=256):
    scores1 = tanh(cat(q, k, -1)) @ w_v                       # [B,S]
    scores2 = softmax(<size-1 axis>) == 1.0 exactly           # path 2 drops out
    p       = softmax(0.5*scores1 + 0.5, axis=S)              # +0.5 shift cancels
    attn    = softmax(where(s < L, p, -1e6), axis=S)          # second softmax on probs
    out     = attn @ v                                        # [B,1,DV]

Per core (8 batch slots): s rows are packed 4-per-partition so DMA runs are
2-4KB contiguous.  scores for a batch live in one [128,16] SBUF tile;
partition-dim reductions go through gpsimd.partition_all_reduce (result is
broadcast to every partition, feeding the next ACT scale directly).  exp()
skips max-subtraction: |0.5*scores1| is bounded by 0.5*sum|w_v| (~6) and the
second softmax's inputs are in (0,1].

attn@v runs with v as the PE stationary operand ([K=128, M=128] halves,
streaming the single attention-weight column) because fp32 LDWEIGHTS ingests
at ~1 elem/cycle while fp32 rhs streaming costs ~2 cycles/col.

Rows with s >= valid_len get exactly zero attention weight (the mask zeroes
them before the second softmax), so v tiles entirely above valid_len are
never loaded or matmul'd.  Batches are sorted by valid_len into slots so one
SPMD program (tile count baked per slot) serves all 8 cores; the program is
rebuilt only when the per-slot tile counts change.

DMA rings: streaming loads (q/k/v) ride the SP HWDGE ring; the tiny
compute-dependent output stores ride GpSimd SWDGE so they never
head-of-line-block the loads.
"""

import math
import os
import sys

import numpy as np

B, S, D, DV = 64, 2048, 128, 256
NCORES = 8
BPC = B // NCORES  # batch slots per core
P = 128            # SBUF partitions
J = 4              # s rows packed per partition per big tile
TT = S // (P * J)  # big s-tiles per batch (4)
C = TT * J         # score columns per batch (16)

_CACHE: dict = {}


def _ensure_import():
    try:
        import concourse.bass  # noqa: F401
        return
    except ImportError:
        pass
    for p in ("/opt/trn_rl_repo", "/root/.axon_site/_ro/trn_rl_repo", "/opt/pypackages"):
        if os.path.isdir(p) and p not in sys.path:
            sys.path.append(p)
    import concourse.bass  # noqa: F401


def _build(slot_tiles):
    """Build + compile the SPMD Bass program for the given per-slot v-tile
    counts (slot_tiles[b] in 1..TT)."""
    from contextlib import ExitStack

    import concourse.bass_isa as bass_isa
    import concourse.tile as tile
    from concourse import bacc, mybir

    f32 = mybir.dt.float32
    Alu = mybir.AluOpType
    Act = mybir.ActivationFunctionType

    nc = bacc.Bacc(
        "TRN2",
        target_bir_lowering=False,
        debug=False,
        enable_asserts=False,
        num_devices=NCORES,
    )

    q = nc.dram_tensor("q", [BPC, S, D], f32, kind="ExternalInput").ap()
    k = nc.dram_tensor("k", [BPC, S, D], f32, kind="ExternalInput").ap()
    v = nc.dram_tensor("v", [BPC, S, DV], f32, kind="ExternalInput").ap()
    lens = nc.dram_tensor("lens", [1, BPC], f32, kind="ExternalInput").ap()
    wv = nc.dram_tensor("wv", [P, 2 * J * D], f32, kind="ExternalInput").ap()
    iota = nc.dram_tensor("iota", [P, C], f32, kind="ExternalInput").ap()
    out = nc.dram_tensor("out", [BPC, 1, DV], f32, kind="ExternalOutput").ap()

    # s = tt*(P*J) + p*J + j
    q_r = q.rearrange("b (tt p j) d -> b tt p j d", p=P, j=J)
    k_r = k.rearrange("b (tt p j) d -> b tt p j d", p=P, j=J)
    v_r = v.rearrange("b (tt p j) dv -> b tt p j dv", p=P, j=J)

    with tile.TileContext(nc) as tc, ExitStack() as ctx:
        n_v_tiles = min(int(sum(slot_tiles)) + TT, 24)  # full v residency + lookahead
        consts = ctx.enter_context(tc.tile_pool(name="consts", bufs=1))
        qk_pool = ctx.enter_context(tc.tile_pool(name="qk", bufs=14))
        th_pool = ctx.enter_context(tc.tile_pool(name="th", bufs=5))
        scr_pool = ctx.enter_context(tc.tile_pool(name="scr", bufs=6))
        v_pool = ctx.enter_context(tc.tile_pool(name="v", bufs=n_v_tiles))
        s1_pool = ctx.enter_context(tc.tile_pool(name="s1", bufs=5))
        sm_pool = ctx.enter_context(tc.tile_pool(name="sm", bufs=8))
        ob_pool = ctx.enter_context(tc.tile_pool(name="ob", bufs=3))
        ps_acc = ctx.enter_context(tc.tile_pool(name="ps_acc", bufs=4, space="PSUM"))

        wv_sb = consts.tile([P, 2 * J * D], f32, tag="wv")
        nc.sync.dma_start(wv_sb[:], wv)
        iota_sb = consts.tile([P, C], f32, tag="iota")
        nc.sync.dma_start(iota_sb[:], iota)
        lens_sb = consts.tile([1, BPC], f32, tag="lens")
        nc.sync.dma_start(lens_sb[:], lens)

        # valid_lens broadcast to every partition: [P, BPC]
        lens_bc = consts.tile([P, BPC], f32, tag="lensbc")
        nc.gpsimd.partition_broadcast(lens_bc[:], lens_sb[:], channels=P)

        def epilogue(acc, rz2b, b):
            ob = ob_pool.tile([1, DV], f32, tag="ob")
            nc.vector.tensor_scalar_mul(ob[:], acc[:], rz2b[0:1, :])
            nc.gpsimd.dma_start(out[b], ob[:])

        def chain(s1, v_tiles, ntt, b):
            """Softmax over S + masked re-softmax + attn@v for slot b.
            Returns the epilogue state (PSUM acc + 1/Z2)."""
            e = sm_pool.tile([P, C], f32, tag="e")
            esum = sm_pool.tile([P, 1], f32, tag="esum")
            nc.scalar.activation(e[:], s1[:], Act.Exp, accum_out=esum[:])
            z1b = sm_pool.tile([P, 1], f32, tag="z1b")
            nc.gpsimd.partition_all_reduce(z1b[:], esum[:], P, bass_isa.ReduceOp.add)
            rz1b = sm_pool.tile([P, 1], f32, tag="rz1b")
            nc.vector.reciprocal(rz1b[:], z1b[:])

            em = sm_pool.tile([P, C], f32, tag="em")
            nc.scalar.activation(em[:], e[:], Act.Exp, scale=rz1b[:])
            w = sm_pool.tile([P, C], f32, tag="w")
            wsum = sm_pool.tile([P, 1], f32, tag="wsum")
            nc.vector.scalar_tensor_tensor(
                out=w[:],
                in0=iota_sb[:],
                scalar=lens_bc[:, b : b + 1],
                in1=em[:],
                op0=Alu.is_lt,
                op1=Alu.mult,
                accum_out=wsum[:],
            )
            z2b = sm_pool.tile([P, 1], f32, tag="z2b")
            nc.gpsimd.partition_all_reduce(z2b[:], wsum[:], P, bass_isa.ReduceOp.add)
            rz2b = sm_pool.tile([P, 1], f32, tag="rz2b")
            nc.vector.reciprocal(rz2b[:], z2b[:])

            nmm = ntt * J
            acc = ps_acc.tile([1, DV], f32, tag="acc")
            for tt in range(ntt):
                for j in range(J):
                    c = tt * J + j
                    nc.tensor.matmul(
                        acc[:],
                        w[:, c : c + 1],
                        v_tiles[tt][:, j * DV : (j + 1) * DV],
                        start=(c == 0),
                        stop=(c == nmm - 1),
                    )
            return acc, rz2b, b

        chain_q = []
        pending_epi = None
        for b in range(BPC):
            ntt = slot_tiles[b]
            s1 = s1_pool.tile([P, C], f32, tag="s1")
            v_tiles = []
            for tt in range(TT):
                # layout [q(j d) | k(j d)]: both DMA dsts are contiguous
                # per partition; compute reads the halves via a strided AP.
                qk = qk_pool.tile([P, J * 2 * D], f32, tag="qk")
                nc.sync.dma_start(
                    qk[:, 0 : J * D].rearrange("p (j d) -> p j d", j=J), q_r[b, tt]
                )
                nc.sync.dma_start(
                    qk[:, J * D : 2 * J * D].rearrange("p (j d) -> p j d", j=J),
                    k_r[b, tt],
                )
                if tt < ntt:
                    vt = v_pool.tile([P, J * DV], f32, tag="v")
                    nc.gpsimd.dma_start(
                        vt[:].rearrange("p (j dv) -> p j dv", j=J), v_r[b, tt]
                    )
                    v_tiles.append(vt)
                th = th_pool.tile([P, J * 2 * D], f32, tag="th")
                nc.scalar.activation(th[:], qk[:], Act.Tanh)
                th5 = th[:].rearrange("p (h j d) -> p j h d", h=2, j=J)
                wv5 = wv_sb[:].rearrange("p (h j d) -> p j h d", h=2, j=J)
                for j in range(J):
                    c = tt * J + j
                    scr = scr_pool.tile([P, 2 * D], f32, tag="scr")
                    # out = (th*0.5 + 0)*wv; accum = row-sum -> 0.5*scores1
                    nc.vector.affine_mul_reduce(
                        out=scr[:].rearrange("p (h d) -> p h d", h=2),
                        accum_out=s1[:, c : c + 1],
                        in0=th5[:, j],
                        in1=wv5[:, j],
                        scale=0.5,
                        bias=0.0,
                    )

            # flush the previous slot's chain after this slot's score block:
            # its inputs are then a full slot old, so these ops never stall
            # an engine queue head.
            if pending_epi is not None:
                epilogue(*pending_epi)
            pending_epi = None
            if len(chain_q) >= 1:
                pending_epi = chain(*chain_q.pop(0))
            chain_q.append((s1, v_tiles, ntt, b))

        if pending_epi is not None:
            epilogue(*pending_epi)
        for st in chain_q:
            epilogue(*chain(*st))

    nc.compile()
    return nc


def _constants():
    iota_np = np.empty((P, C), np.float32)
    for tt in range(TT):
        for j in range(J):
            iota_np[:, tt * J + j] = tt * (P * J) + np.arange(P) * J + j
    return (iota_np,)


def _get_built(slot_tiles):
    slot_tiles = tuple(int(t) for t in slot_tiles)
    key = ("nc", slot_tiles)
    if key not in _CACHE:
        _ensure_import()
        _CACHE[key] = _build(slot_tiles)
    if "consts" not in _CACHE:
        _CACHE["consts"] = _constants()
    return _CACHE[key], _CACHE["consts"]


def plan(valid_lens):
    """Sort batches by valid_len (desc) into (slot, core) and derive the
    per-slot v-tile counts baked into the SPMD program."""
    vl = np.asarray(valid_lens).reshape(B).astype(np.int64)
    order = np.argsort(-vl, kind="stable")  # batch index for (slot*NCORES + core)
    slot_tiles = []
    for kslot in range(BPC):
        group = vl[order[kslot * NCORES : (kslot + 1) * NCORES]]
        slot_tiles.append(max(1, math.ceil(int(group.max()) / (P * J))))
    return order, tuple(slot_tiles)


def run(nc, in_maps, trace=False, **kwargs):
    from concourse.bass_utils import run_bass_kernel_spmd

    return run_bass_kernel_spmd(
        nc, in_maps, core_ids=list(range(NCORES)), trace=trace, **kwargs
    )


def make_in_maps(queries, keys, values, valid_lens, w_v, order):
    q = np.asarray(queries, np.float32)
    k = np.asarray(keys, np.float32)
    v = np.asarray(values, np.float32)
    vl = np.asarray(valid_lens).astype(np.float32).reshape(B)
    wv_row = np.asarray(w_v, np.float32).reshape(2 * D)

    (iota_np,) = _CACHE.get("consts") or _constants()
    # match the th tile layout (h j d): per half, w_v repeats across j
    wv_line = np.concatenate([np.tile(wv_row[:D], J), np.tile(wv_row[D:], J)])
    wv_bcast = np.ascontiguousarray(np.broadcast_to(wv_line, (P, 2 * J * D)))

    in_maps = []
    for core in range(NCORES):
        batches = [int(order[kslot * NCORES + core]) for kslot in range(BPC)]
        in_maps.append(
            {
                "q": np.ascontiguousarray(q[batches]),
                "k": np.ascontiguousarray(k[batches]),
                "v": np.ascontiguousarray(v[batches]),
                "lens": np.ascontiguousarray(vl[batches].reshape(1, BPC)),
                "wv": wv_bcast,
                "iota": iota_np,
            }
        )
    return in_maps


def kernel(queries, keys, values, valid_lens, w_v, w2, w_v2_w, w_v2_b, **_unused):
    # w2 / w_v2_w / w_v2_b feed a softmax over a size-1 axis, which is
    # identically 1.0; the 0.5*1.0 blend term is a constant shift that a
    # softmax ignores, so those parameters cannot affect the output.
    _ensure_import()
    order, slot_tiles = plan(valid_lens)
    nc, _ = _get_built(slot_tiles)
    in_maps = make_in_maps(queries, keys, values, valid_lens, w_v, order)
    res = run(nc, in_maps)
    out = np.empty((B, 1, DV), np.float32)
    for core in range(NCORES):
        for kslot in range(BPC):
            out[int(order[kslot * NCORES + core])] = res.results[core]["out"][kslot]
    return out

